# revision 1
# baseline (speedup 1.0000x reference)
# Multi-head causal self-attention with RoPE on 8 NeuronCores (Trainium2).
#
# Sharding: zero-communication data parallel. Core c handles batch b = c//2
# and a balanced half of that batch's queries (half = c%2):
#   half 0 -> query rows [0:512) u [1536:2048)   (early + late stripe)
#   half 1 -> query rows [512:1536)
# Both halves attend over the full 2048-token K/V of their batch (K/V
# projection is duplicated across the pair — the price of zero comms).
# Every core runs the same program (SPMD); per-core differences live purely
# in the input data (gathered query slices, RoPE tables, causal masks).
#
# Layouts (on chip, bf16 compute / f32 accumulate):
#   q^T, k^T  [128 part = head-pair dims, tokens]   d-major for S^T matmuls
#   V         [128 part = tokens, dims]             token-major, +ones col for
#                                                   the softmax denominator
#   S^T tiles [128 j-tokens, 512 queries]           softmax along PARTITION j
#                                                   via matmul-with-ones —
#                                                   no transposes anywhere.
# RoPE uses an "evens-then-odds" permuted head layout (baked into Wq/Wk
# columns host-side) so the rotation partner is a fixed +-32 partition shift.

import sys

import numpy as np
import ml_dtypes

for _p in ("/opt/trn_rl_repo",):
    try:
        import concourse.bass  # noqa: F401
        break
    except ImportError:
        sys.path.insert(0, _p)

import concourse.bass as bass
import concourse.tile as tile
from concourse import mybir
from concourse.bass_utils import run_bass_kernel_spmd

B, T, D, H, DH = 4, 2048, 1024, 16, 64
THETA = 10000.0
NCORES = 8
P = 128
OC = 8    # 128-wide output-dim chunks (head pairs)
DC = 8    # 128-wide input-dim chunks
NQ = 1024  # queries per core
BLK = 512  # query block width
JA, JB = 8, 16  # j-tiles (128 kv tokens each) per block A / B

f32 = mybir.dt.float32
bf16 = mybir.dt.bfloat16
BF = ml_dtypes.bfloat16


# ---------------------------------------------------------------- host prep

def _perm():
    """Column permutation: within each head's 64 dims, evens then odds."""
    p = np.empty(D, np.int64)
    for h in range(H):
        for m in range(32):
            p[h * 64 + m] = h * 64 + 2 * m
            p[h * 64 + 32 + m] = h * 64 + 2 * m + 1
    return p


def _qpos(core):
    half = core % 2
    if half == 0:
        return np.concatenate([np.arange(0, 512), np.arange(1536, 2048)])
    return np.arange(512, 1536)


def _rope_tables(pos):
    """cos/sin tables [128, len(pos)] for the permuted (evens-first) layout.

    Slot p within a 64-dim head: m = p % 64; freq index = m % 32; the
    rotation partner is p XOR 32 (within the head); sign of the sin term is
    -1 for m < 32, +1 for m >= 32.
    """
    inv = THETA ** (-(np.arange(0, DH, 2, dtype=np.float64) / DH))  # [32]
    m = np.arange(P) % 64
    fi = m % 32
    ang = pos[None, :].astype(np.float64) * inv[fi][:, None]  # [128, L]
    cos = np.cos(ang)
    sin = np.sin(ang) * np.where(m < 32, -1.0, 1.0)[:, None]
    return cos.astype(np.float32), sin.astype(np.float32)


def _masks(core):
    """maskA [8,128,512], maskB [8,128,512] (j-tiles 8..15) for this core."""
    qp = _qpos(core)
    qa, qb = qp[:BLK], qp[BLK:]
    jj = np.arange(P)
    mA = np.empty((JA, P, BLK), np.float32)
    for jt in range(JA):
        mA[jt] = ((jt * P + jj)[:, None] <= qa[None, :]).astype(np.float32)
    mB = np.empty((8, P, BLK), np.float32)
    for jt in range(8, 16):
        mB[jt - 8] = ((jt * P + jj)[:, None] <= qb[None, :]).astype(np.float32)
    return mA, mB


def host_prep(x, Wq, bq, Wk, bk, Wv, bv, Wo, bo):
    """Build the 8 per-core input dicts (numpy, bf16 unless noted)."""
    perm = _perm()
    WqT = np.ascontiguousarray(Wq.T[:, perm]).astype(BF)
    WkT = np.ascontiguousarray(Wk.T[:, perm]).astype(BF)
    WvT = np.ascontiguousarray(Wv.T).astype(BF)
    WoT = np.ascontiguousarray(Wo.T).astype(BF)
    bqp = bq[perm].reshape(1, D).astype(BF)
    bkp = bk[perm].reshape(1, D).astype(BF)
    bvp = bv.reshape(1, D).astype(BF)
    bop = bo.reshape(1, D).astype(BF)
    ck, sk = _rope_tables(np.arange(T))
    in_maps = []
    for c in range(NCORES):
        b = c // 2
        qp = _qpos(c)
        cq, sq = _rope_tables(qp)
        mA, mB = _masks(c)
        xb = x[b]  # [T, D]
        in_maps.append({
            "xT": np.ascontiguousarray(xb.T).astype(BF),
            "xqT": np.ascontiguousarray(xb[qp].T).astype(BF),
            "WqT": WqT, "WkT": WkT, "WvT": WvT, "WoT": WoT,
            "bq": bqp, "bk": bkp, "bv": bvp, "bo": bop,
            "cq": cq.astype(BF), "sq": sq.astype(BF),
            "ck": ck.astype(BF), "sk": sk.astype(BF),
            "mA": mA.astype(BF), "mB": mB.astype(BF),
        })
    return in_maps


def assemble(results):
    y = np.empty((B, T, D), np.float32)
    for c in range(NCORES):
        y[c // 2, _qpos(c), :] = results[c]["out"]
    return y


# ------------------------------------------------------------- device build

def _legalize_waits(nc, max_waits=1):
    """Limit every instruction to one sync-wait command.

    Walrus's per-instruction structs encode a single sync wait; Tile can
    emit more. For any instruction with k > 1 waits, insert k-1 nops on
    the same engine immediately before it, each carrying one wait —
    position-preserving, so semantics are unchanged.
    """
    eng_obj = {
        mybir.EngineType.PE: nc.tensor,
        mybir.EngineType.Activation: nc.scalar,
        mybir.EngineType.DVE: nc.vector,
        mybir.EngineType.Pool: nc.gpsimd,
        mybir.EngineType.SP: nc.sync,
    }
    fn = nc.m.functions[0]
    for blk in fn.blocks:
        insts = list(blk.instructions)
        new = []
        for inst in insts:
            si = inst.sync_info
            nw = len(si.on_wait) if si is not None else 0
            if nw > max_waits:
                for w in si.on_wait[: nw - max_waits]:
                    eng_obj[inst.engine].nop()
                    nop = fn.blocks[-1].instructions[-1]
                    fn.blocks[-1].instructions = \
                        fn.blocks[-1].instructions[:-1]
                    nop.sync_info = mybir.SyncInfo(on_wait=[w], on_update=[])
                    new.append(nop)
                inst.sync_info = mybir.SyncInfo(
                    on_wait=list(si.on_wait[nw - max_waits:]),
                    on_update=list(si.on_update))
            new.append(inst)
        blk.instructions = new


def build_nc(use_bias):
    from contextlib import ExitStack

    nc = bass.Bass("TRN2", target_bir_lowering=False, debug=False,
                   num_devices=NCORES)
    Exp = mybir.ActivationFunctionType.Exp

    xT = nc.dram_tensor("xT", [D, T], bf16, kind="ExternalInput").ap()
    xqT = nc.dram_tensor("xqT", [D, NQ], bf16, kind="ExternalInput").ap()
    WqT = nc.dram_tensor("WqT", [D, D], bf16, kind="ExternalInput").ap()
    WkT = nc.dram_tensor("WkT", [D, D], bf16, kind="ExternalInput").ap()
    WvT = nc.dram_tensor("WvT", [D, D], bf16, kind="ExternalInput").ap()
    WoT = nc.dram_tensor("WoT", [D, D], bf16, kind="ExternalInput").ap()
    if use_bias:
        bq_d = nc.dram_tensor("bq", [1, D], bf16, kind="ExternalInput").ap()
        bk_d = nc.dram_tensor("bk", [1, D], bf16, kind="ExternalInput").ap()
        bv_d = nc.dram_tensor("bv", [1, D], bf16, kind="ExternalInput").ap()
        bo_d = nc.dram_tensor("bo", [1, D], bf16, kind="ExternalInput").ap()
    cq_d = nc.dram_tensor("cq", [P, NQ], bf16, kind="ExternalInput").ap()
    sq_d = nc.dram_tensor("sq", [P, NQ], bf16, kind="ExternalInput").ap()
    ck_d = nc.dram_tensor("ck", [P, T], bf16, kind="ExternalInput").ap()
    sk_d = nc.dram_tensor("sk", [P, T], bf16, kind="ExternalInput").ap()
    mA_d = nc.dram_tensor("mA", [JA, P, BLK], bf16, kind="ExternalInput").ap()
    mB_d = nc.dram_tensor("mB", [8, P, BLK], bf16, kind="ExternalInput").ap()
    out_d = nc.dram_tensor("out", [NQ, D], f32, kind="ExternalOutput").ap()
    den_d = nc.dram_tensor("den_scratch", [32, BLK], f32)

    with tile.TileContext(nc) as tc, ExitStack() as ctx:
        big = ctx.enter_context(tc.tile_pool(name="big", bufs=1))
        const = ctx.enter_context(tc.tile_pool(name="const", bufs=1))
        ph1_stack = ExitStack()
        ph1 = ph1_stack.enter_context(tc.tile_pool(name="ph1", bufs=1))
        rpool = ph1_stack.enter_context(tc.tile_pool(name="rp", bufs=2))
        psmm = ph1_stack.enter_context(
            tc.tile_pool(name="psmm", bufs=3, space="PSUM"))

        # ---- persistent SBUF tensors (per-dc tiles so compute starts
        # as soon as the first chunks land)
        def load_rows(src, L, tagp):
            tiles = []
            for dc in range(DC):
                t = ph1.tile([P, L], bf16, tag=f"{tagp}{dc}")
                nc.sync.dma_start(t, src[dc * P:(dc + 1) * P, :])
                tiles.append(t)
            return tiles
        x_s = load_rows(xT, T, "x_s")
        xq_s = load_rows(xqT, NQ, "xq_s")
        wv_s = load_rows(WvT, D, "wv_s")
        wq_s = load_rows(WqT, D, "wq_s")
        wk_s = load_rows(WkT, D, "wk_s")
        qfin = big.tile([P, OC, NQ], bf16, tag="qfin")
        kfin = big.tile([P, OC, T], bf16, tag="kfin")
        vaug = big.tile([P, 16, H, 65], bf16, tag="vaug")
        nc.vector.memset(vaug[:, :, :, 64:65], 1.0)

        cq_s = const.tile([P, NQ], bf16, tag="cq")
        nc.sync.dma_start(cq_s, cq_d)
        sq_s = const.tile([P, NQ], bf16, tag="sq")
        nc.sync.dma_start(sq_s, sq_d)
        ck_s = const.tile([P, T], bf16, tag="ck")
        nc.sync.dma_start(ck_s, ck_d)
        sk_s = const.tile([P, T], bf16, tag="sk")
        nc.sync.dma_start(sk_s, sk_d)
        if use_bias:
            bq_s = const.tile([1, D], bf16, tag="bq")
            nc.sync.dma_start(bq_s, bq_d)
            bk_s = const.tile([1, D], bf16, tag="bk")
            nc.sync.dma_start(bk_s, bk_d)
            bv_s = const.tile([1, D], bf16, tag="bv")
            nc.sync.dma_start(bv_s, bv_d)
            bo_s = const.tile([1, D], bf16, tag="bo")
            nc.sync.dma_start(bo_s, bo_d)
            ones512 = const.tile([1, BLK], bf16, tag="ones512")
            nc.vector.memset(ones512, 1.0)
            onesb = const.tile([1, P], bf16, tag="onesb")
            nc.vector.memset(onesb, 1.0)

        def proj(ps, w_tiles, osl, rhs_s, t_lo, use_b, b_s, oc):
            for dc in range(DC):
                nc.tensor.matmul(ps, w_tiles[dc][:, osl],
                                 rhs_s[dc][:, t_lo:t_lo + BLK],
                                 start=(dc == 0),
                                 stop=(dc == DC - 1 and not use_b))
            if use_b:
                nc.tensor.matmul(ps, b_s[:, oc * P:(oc + 1) * P], ones512,
                                 start=False, stop=True)

        def rope(fin, oc, t_c, cos_s, sin_s):
            # rotate fin[:, oc, t_c*BLK:(t_c+1)*BLK] in place (one producer)
            sl = slice(t_c * BLK, (t_c + 1) * BLK)
            sw = rpool.tile([P, BLK], bf16, tag="sw")
            for (a, src) in ((0, 32), (32, 0), (64, 96), (96, 64)):
                nc.gpsimd.dma_start(sw[a:a + 32, :], fin[src:src + 32, oc, sl])
            t1 = rpool.tile([P, BLK], bf16, tag="t1")
            t2 = rpool.tile([P, BLK], bf16, tag="t2")
            nc.vector.tensor_mul(t1, fin[:, oc, sl], cos_s[:, sl])
            nc.vector.tensor_mul(t2, sw, sin_s[:, sl])
            nc.vector.tensor_add(fin[:, oc, sl], t1, t2)

        # ---- Q/K projections + RoPE
        for oc in range(OC):
            osl = slice(oc * P, (oc + 1) * P)
            for t_c in range(NQ // BLK):
                ps = psmm.tile([P, BLK], f32, tag="mm")
                proj(ps, wq_s, osl, xq_s, t_c * BLK, use_bias,
                     bq_s if use_bias else None, oc)
                nc.any.tensor_copy(qfin[:, oc, t_c * BLK:(t_c + 1) * BLK], ps)
                rope(qfin, oc, t_c, cq_s, sq_s)
            for t_c in range(T // BLK):
                ps = psmm.tile([P, BLK], f32, tag="mm")
                proj(ps, wk_s, osl, x_s, t_c * BLK, use_bias,
                     bk_s if use_bias else None, oc)
                nc.any.tensor_copy(kfin[:, oc, t_c * BLK:(t_c + 1) * BLK], ps)
                rope(kfin, oc, t_c, ck_s, sk_s)

        # ---- V projection (token-major, straight into vaug)
        for tt in range(16):
            for oc2 in range(2):
                ps = psmm.tile([P, BLK], f32, tag="mm")
                for dc in range(DC):
                    nc.tensor.matmul(ps, x_s[dc][:, tt * P:(tt + 1) * P],
                                     wv_s[dc][:, oc2 * BLK:(oc2 + 1) * BLK],
                                     start=(dc == 0),
                                     stop=(dc == DC - 1 and not use_bias))
                if use_bias:
                    nc.tensor.matmul(ps, onesb,
                                     bv_s[:, oc2 * BLK:(oc2 + 1) * BLK],
                                     start=False, stop=True)
                nc.any.tensor_copy(
                    vaug[:, tt, oc2 * 8:(oc2 + 1) * 8, 0:64], ps)

        # ---- phase 2: close projection pools, open attention pools
        ph1_stack.close()
        att_stack = ExitStack()
        psst = att_stack.enter_context(
            tc.tile_pool(name="psst", bufs=1, space="PSUM"))
        pso = att_stack.enter_context(
            tc.tile_pool(name="pso", bufs=2, space="PSUM"))
        ph2 = ctx.enter_context(tc.tile_pool(name="ph2", bufs=1))
        ptp = ctx.enter_context(tc.tile_pool(name="ptp", bufs=2))
        rbp = ctx.enter_context(tc.tile_pool(name="rbp", bufs=2))
        outp = ctx.enter_context(tc.tile_pool(name="outp", bufs=3))

        mA_s = ph2.tile([P, JA, BLK], bf16, tag="mA")
        nc.sync.dma_start(mA_s, mA_d.rearrange("jt p i -> p jt i"))
        mB_s = ph2.tile([P, 8, BLK], bf16, tag="mB")
        nc.sync.dma_start(mB_s, mB_d.rearrange("jt p i -> p jt i"))
        den_sb = ph2.tile([32, BLK], f32, tag="den")
        den_r = ph2.tile([32, BLK], f32, tag="denr")
        ctxu = ph2.tile([P, OC, NQ], bf16, tag="ctxu")

        # ---- attention
        for oc in range(OC):
            h0, h1 = 2 * oc, 2 * oc + 1
            for blk in range(2):
                J = JA if blk == 0 else JB
                q_lo = blk * BLK
                opsA = pso.tile([P, BLK], f32, tag="oA")
                opsB = pso.tile([P, BLK], f32, tag="oB")
                for g in range(J // 2):
                    sA = psst.tile([P, 2 * BLK], f32, tag="sA")
                    sB = psst.tile([P, 2 * BLK], f32, tag="sB")
                    for dj in range(2):
                        jt = 2 * g + dj
                        nc.tensor.matmul(
                            sA[:, dj * BLK:(dj + 1) * BLK],
                            kfin[0:64, oc, jt * P:(jt + 1) * P],
                            qfin[0:64, oc, q_lo:q_lo + BLK],
                            start=True, stop=True, tile_position=(0, 0))
                        nc.tensor.matmul(
                            sB[:, dj * BLK:(dj + 1) * BLK],
                            kfin[64:128, oc, jt * P:(jt + 1) * P],
                            qfin[64:128, oc, q_lo:q_lo + BLK],
                            start=True, stop=True, tile_position=(64, 0))
                    pA = ptp.tile([P, 2 * BLK], bf16, tag="pA")
                    pB = ptp.tile([P, 2 * BLK], bf16, tag="pB")
                    nc.scalar.activation(pA, sA, Exp, scale=0.125)
                    nc.scalar.activation(pB, sB, Exp, scale=0.125)
                    for dj in range(2):
                        jt = 2 * g + dj
                        msk = None
                        if blk == 0:
                            msk = mA_s[:, jt, :]
                        elif jt >= 8:
                            msk = mB_s[:, jt - 8, :]
                        sl = slice(dj * BLK, (dj + 1) * BLK)
                        if msk is not None:
                            nc.vector.tensor_mul(pA[:, sl], pA[:, sl], msk)
                            nc.vector.tensor_mul(pB[:, sl], pB[:, sl], msk)
                        nc.tensor.matmul(opsA[0:65, :], vaug[:, jt, h0, :],
                                         pA[:, sl], start=(jt == 0),
                                         stop=(jt == J - 1))
                        nc.tensor.matmul(opsB[0:65, :], vaug[:, jt, h1, :],
                                         pB[:, sl], start=(jt == 0),
                                         stop=(jt == J - 1))
                row = oc * 4 + blk * 2
                nc.vector.tensor_copy(ctxu[0:64, oc, q_lo:q_lo + BLK],
                                       opsA[0:64, :])
                nc.vector.tensor_copy(ctxu[64:128, oc, q_lo:q_lo + BLK],
                                      opsB[0:64, :])
                for (r, ops) in ((row, opsA), (row + 1, opsB)):
                    stg = rbp.tile([1, BLK], f32, tag="dstage")
                    nc.vector.tensor_copy(stg, ops[64:65, :])
                    nc.gpsimd.dma_start(den_sb[r:r + 1, :], stg)

        # ---- normalize: 1/den broadcast via DRAM round-trip
        nc.vector.reciprocal(den_r, den_sb)
        nc.sync.dma_start(den_d.ap(), den_r)
        for oc in range(OC):
            for blk in range(2):
                row = oc * 4 + blk * 2
                q_lo = blk * BLK
                rb = rbp.tile([P, BLK], f32, tag="rb")
                for (hh, r) in ((0, row), (64, row + 1)):
                    sl = den_d.ap()[r:r + 1, :]
                    src = bass.AP(tensor=sl.tensor, offset=sl.offset,
                                  ap=[[0, 64]] + sl.ap[1:])
                    nc.gpsimd.dma_start(rb[hh:hh + 64, :], src)
                nc.vector.tensor_mul(ctxu[:, oc, q_lo:q_lo + BLK],
                                     ctxu[:, oc, q_lo:q_lo + BLK], rb)

        # ---- output projection
        att_stack.close()
        psmm = ctx.enter_context(
            tc.tile_pool(name="psmm2", bufs=3, space="PSUM"))
        wo_s = []
        for dc in range(DC):
            t = ph2.tile([P, D], bf16, tag=f"wo_s{dc}")
            nc.sync.dma_start(t, WoT[dc * P:(dc + 1) * P, :])
            wo_s.append(t)
        for tcp in range(8):
            for oc2 in range(2):
                ps = psmm.tile([P, BLK], f32, tag="mm")
                for dc in range(DC):
                    nc.tensor.matmul(ps, ctxu[:, dc, tcp * P:(tcp + 1) * P],
                                     wo_s[dc][:, oc2 * BLK:(oc2 + 1) * BLK],
                                     start=(dc == 0),
                                     stop=(dc == DC - 1 and not use_bias))
                if use_bias:
                    nc.tensor.matmul(ps, onesb,
                                     bo_s[:, oc2 * BLK:(oc2 + 1) * BLK],
                                     start=False, stop=True)
                ot = outp.tile([P, BLK], f32, tag="ot")
                nc.any.tensor_copy(ot, ps)
                nc.sync.dma_start(
                    out_d[tcp * P:(tcp + 1) * P,
                          oc2 * BLK:(oc2 + 1) * BLK], ot)
    _legalize_waits(nc)
    return nc


# ------------------------------------------------------------------- entry

def kernel(x, Wq, bq, Wk, bk, Wv, bv, Wo, bo):
    x = np.asarray(x, np.float32)
    Wq, bq = np.asarray(Wq, np.float32), np.asarray(bq, np.float32)
    Wk, bk = np.asarray(Wk, np.float32), np.asarray(bk, np.float32)
    Wv, bv = np.asarray(Wv, np.float32), np.asarray(bv, np.float32)
    Wo, bo = np.asarray(Wo, np.float32), np.asarray(bo, np.float32)
    use_bias = bool(any(np.any(b) for b in (bq, bk, bv, bo)))
    in_maps = host_prep(x, Wq, bq, Wk, bk, Wv, bv, Wo, bo)
    if not use_bias:
        for m in in_maps:
            for k in ("bq", "bk", "bv", "bo"):
                m.pop(k)
    nc = build_nc(use_bias)
    res = run_bass_kernel_spmd(nc, in_maps, list(range(NCORES))).results
    return assemble(res)



# revision 16
# speedup vs baseline: 1.2992x; 1.2992x over previous
# Multi-head causal self-attention with RoPE on 8 NeuronCores (Trainium2).
#
# Sharding: TP-2 x DP-4, zero device communication. Core c handles batch
# b = c//2 and head group g = c%2 (heads 8g..8g+7) over ALL 2048 tokens.
# Each core computes a PARTIAL output projection (its 512 head-dims rows of
# Wo^T); the host sums the two partials per batch during unsharding.
# This removes the K/V-projection duplication of a pure-DP split and makes
# the causal tiling exact (no fully-masked score tiles).
#
# Pipeline (single phase, software-pipelined so Scalar exp overlaps PE):
#   K proj (+RoPE), V proj  ->  for qb in 0..3:
#     Q proj(qb) (+RoPE); for oc in 0..3: scores/exp/mask/attnV over the
#     causal j-tiles; per-(oc,qb) normalize via selector-matmul broadcast
#     of 1/den (no DRAM round trip); out-proj(qb) + output DMA.
#
# Layouts (on chip, bf16 compute / f32 accumulate):
#   q^T, k^T  [128 part = head-pair dims, tokens]    d-major for S^T matmuls
#   vaug      [128 part = tokens, 16 tt, 8 h, 66]    cols = [ones|V(64)|ones]
#             h-even uses cols 1:66 (ctx rows 0..63, den row 64);
#             h-odd  uses cols 0:65 (den row 63, ctx rows 64..127) so the
#             head-pair context lands lane-aligned in one 128-part tile.
#   S^T tiles [128 j-tokens, 1024] = h0|h1 halves    softmax along PARTITION
#             via matmul-with-ones; one exp instr covers both heads.
# RoPE uses an "evens-then-odds" permuted head layout (baked into Wq/Wk
# columns host-side) so the rotation partner is a fixed +-32 partition shift.

import sys

import numpy as np
import ml_dtypes

for _p in ("/opt/trn_rl_repo",):
    try:
        import concourse.bass  # noqa: F401
        break
    except ImportError:
        sys.path.insert(0, _p)

import concourse.bass as bass
import concourse.tile as tile
from concourse import mybir
from concourse.bass_utils import run_bass_kernel_spmd

B, T, D, H, DH = 4, 2048, 1024, 16, 64
THETA = 10000.0
NCORES = 8
P = 128
DG = 512   # head dims per core (8 heads)
OC = 4     # 128-wide head-pair chunks per core
DC = 8     # 128-wide input-dim chunks
BLK = 512  # query block width
NQB = 4    # query blocks
NTT = 16   # 128-token tiles

f32 = mybir.dt.float32
bf16 = mybir.dt.bfloat16
BF = ml_dtypes.bfloat16


# ---------------------------------------------------------------- host prep

def _perm():
    """Column permutation: within each head's 64 dims, evens then odds."""
    p = np.empty(D, np.int64)
    for h in range(H):
        for m in range(32):
            p[h * 64 + m] = h * 64 + 2 * m
            p[h * 64 + 32 + m] = h * 64 + 2 * m + 1
    return p


def _rope_tables():
    """cos/sin tables [128, T] for the permuted (evens-first) layout."""
    inv = THETA ** (-(np.arange(0, DH, 2, dtype=np.float64) / DH))  # [32]
    m = np.arange(P) % 64
    fi = m % 32
    pos = np.arange(T)
    ang = pos[None, :].astype(np.float64) * inv[fi][:, None]  # [128, T]
    cos = np.cos(ang)
    sin = np.sin(ang) * np.where(m < 64 // 2, -1.0, 1.0)[:, None]
    return cos.astype(np.float32), sin.astype(np.float32)


def _masks():
    """Staircase masks [4, 128, 1024] (identical h0|h1 halves).

    For the diagonal j-tile at kv offset 128*jt within a 512-query block:
    m[jt, r, c] = 1 iff 128*jt + r <= (c % 512).
    """
    mk = np.zeros((4, P, 2 * BLK), np.float32)
    r = np.arange(P)[:, None]
    cl = (np.arange(2 * BLK) % BLK)[None, :]
    for jt in range(4):
        mk[jt] = (128 * jt + r <= cl).astype(np.float32)
    return mk


def host_prep(x, Wq, bq, Wk, bk, Wv, bv, Wo, bo):
    """Build the 8 per-core input dicts (numpy, bf16)."""
    perm = _perm()
    WqTp = np.ascontiguousarray(Wq.T[:, perm]).astype(BF)
    WkTp = np.ascontiguousarray(Wk.T[:, perm]).astype(BF)
    WvT = np.ascontiguousarray(Wv.T).astype(BF)
    WoT = np.ascontiguousarray(Wo.T).astype(BF)
    bqp = bq[perm].astype(np.float32)
    bkp = bk[perm].astype(np.float32)
    ck, sk = _rope_tables()
    mk = _masks().reshape(4 * P, 2 * BLK)
    # sel[r, oc, c]: broadcast-selector for the per-(oc) 1/den matmul:
    # row 2oc -> cols 0:64 (head h0), row 2oc+1 -> cols 64:128 (head h1)
    sel = np.zeros((8, OC, P), np.float32)
    for oc in range(OC):
        sel[2 * oc, oc, 0:64] = 1.0
        sel[2 * oc + 1, oc, 64:128] = 1.0
    sel = sel.reshape(8, OC * P)
    in_maps = []
    for c in range(NCORES):
        b, g = c // 2, c % 2
        gs = slice(DG * g, DG * (g + 1))
        in_maps.append({
            "xT": np.ascontiguousarray(x[b].T).astype(BF),
            "WqT": WqTp[:, gs], "WkT": WkTp[:, gs],
            "WvT": np.ascontiguousarray(WvT[:, gs]),
            "WoT": np.ascontiguousarray(WoT[gs, :]),
            "bq": bqp[gs].reshape(1, DG).astype(BF),
            "bk": bkp[gs].reshape(1, DG).astype(BF),
            "bv": bv[gs].reshape(1, DG).astype(BF),
            # host sums the two partials, so each adds half of bo
            "bo": (0.5 * bo).reshape(1, D).astype(BF),
            "ck": ck.astype(BF), "sk": sk.astype(BF),
            "mk": mk.astype(BF), "sel": sel.astype(BF),
        })
    return in_maps


def assemble(results):
    y = np.empty((B, T, D), np.float32)
    for b in range(B):
        y[b] = results[2 * b]["out"] + results[2 * b + 1]["out"]
    return y


# ------------------------------------------------------------- device build

def _legalize_waits(nc, max_waits=1):
    """Limit every instruction to one sync-wait command.

    Walrus's per-instruction structs encode a single sync wait; Tile can
    emit more. For any instruction with k > 1 waits, insert k-1 nops on
    the same engine immediately before it, each carrying one wait —
    position-preserving, so semantics are unchanged.
    """
    eng_obj = {
        mybir.EngineType.PE: nc.tensor,
        mybir.EngineType.Activation: nc.scalar,
        mybir.EngineType.DVE: nc.vector,
        mybir.EngineType.Pool: nc.gpsimd,
        mybir.EngineType.SP: nc.sync,
    }
    fn = nc.m.functions[0]
    for blk in fn.blocks:
        insts = list(blk.instructions)
        new = []
        for inst in insts:
            si = inst.sync_info
            nw = len(si.on_wait) if si is not None else 0
            if nw > max_waits:
                for w in si.on_wait[: nw - max_waits]:
                    eng_obj[inst.engine].nop()
                    nop = fn.blocks[-1].instructions[-1]
                    fn.blocks[-1].instructions = \
                        fn.blocks[-1].instructions[:-1]
                    nop.sync_info = mybir.SyncInfo(on_wait=[w], on_update=[])
                    new.append(nop)
                inst.sync_info = mybir.SyncInfo(
                    on_wait=list(si.on_wait[nw - max_waits:]),
                    on_update=list(si.on_update))
            new.append(inst)
        blk.instructions = new


def build_nc(use_bias):
    from contextlib import ExitStack

    nc = bass.Bass("TRN2", target_bir_lowering=False, debug=False,
                   num_devices=NCORES)
    Exp = mybir.ActivationFunctionType.Exp
    Ln = mybir.ActivationFunctionType.Ln

    xT = nc.dram_tensor("xT", [D, T], bf16, kind="ExternalInput").ap()
    WqT = nc.dram_tensor("WqT", [D, DG], bf16, kind="ExternalInput").ap()
    WkT = nc.dram_tensor("WkT", [D, DG], bf16, kind="ExternalInput").ap()
    WvT = nc.dram_tensor("WvT", [D, DG], bf16, kind="ExternalInput").ap()
    WoT = nc.dram_tensor("WoT", [DG, D], bf16, kind="ExternalInput").ap()
    if use_bias:
        bq_d = nc.dram_tensor("bq", [1, DG], bf16, kind="ExternalInput").ap()
        bk_d = nc.dram_tensor("bk", [1, DG], bf16, kind="ExternalInput").ap()
        bv_d = nc.dram_tensor("bv", [1, DG], bf16, kind="ExternalInput").ap()
        bo_d = nc.dram_tensor("bo", [1, D], bf16, kind="ExternalInput").ap()
    ck_d = nc.dram_tensor("ck", [P, T], bf16, kind="ExternalInput").ap()
    sk_d = nc.dram_tensor("sk", [P, T], bf16, kind="ExternalInput").ap()
    mk_d = nc.dram_tensor("mk", [4 * P, 2 * BLK], bf16,
                          kind="ExternalInput").ap()
    sel_d = nc.dram_tensor("sel", [8, OC * P], bf16,
                           kind="ExternalInput").ap()
    out_d = nc.dram_tensor("out", [T, D], f32, kind="ExternalOutput").ap()

    with tile.TileContext(nc) as tc, ExitStack() as ctx:
        big = ctx.enter_context(tc.tile_pool(name="big", bufs=1))
        const = ctx.enter_context(tc.tile_pool(name="const", bufs=1))
        rpool = ctx.enter_context(tc.tile_pool(name="rp", bufs=2))
        ppool = ctx.enter_context(tc.tile_pool(name="pp", bufs=3))
        npool = ctx.enter_context(tc.tile_pool(name="np", bufs=2))
        outp = ctx.enter_context(tc.tile_pool(name="outp", bufs=3))
        pssc = ctx.enter_context(
            tc.tile_pool(name="pssc", bufs=2, space="PSUM"))
        psacc = ctx.enter_context(
            tc.tile_pool(name="psacc", bufs=1, space="PSUM"))
        psmm = ctx.enter_context(
            tc.tile_pool(name="psmm", bufs=2, space="PSUM"))

        # ---- constants (small first, scalar queue is idle early)
        sel_s = const.tile([8, OC, P], bf16, tag="sel")
        nc.scalar.dma_start(sel_s, sel_d.rearrange("r (oc p) -> r oc p", p=P))
        mk_s = const.tile([P, 4, 2 * BLK], bf16, tag="mk")
        nc.scalar.dma_start(
            mk_s, mk_d.rearrange("(jt p) i -> p jt i", p=P))
        ck_s = const.tile([P, T], bf16, tag="ck")
        nc.scalar.dma_start(ck_s, ck_d)
        sk_s = const.tile([P, T], bf16, tag="sk")
        nc.scalar.dma_start(sk_s, sk_d)
        if use_bias:
            bq_s = const.tile([1, DG], bf16, tag="bq")
            nc.scalar.dma_start(bq_s, bq_d)
            bk_s = const.tile([1, DG], bf16, tag="bk")
            nc.scalar.dma_start(bk_s, bk_d)
            bv_s = const.tile([1, DG], bf16, tag="bv")
            nc.scalar.dma_start(bv_s, bv_d)
            bo_s = const.tile([1, D], bf16, tag="bo")
            nc.scalar.dma_start(bo_s, bo_d)
            ones512 = const.tile([1, BLK], bf16, tag="ones512")
            nc.vector.memset(ones512, 1.0)
            onesb = const.tile([1, P], bf16, tag="onesb")
            nc.vector.memset(onesb, 1.0)

        # ---- weights: wk first (K proj runs first), then others
        def load_w(src, n_in, n_col, tagp, q):
            tiles = []
            for dc in range(n_in // P):
                t = big.tile([P, n_col], bf16, tag=f"{tagp}{dc}")
                q(t, src[dc * P:(dc + 1) * P, :])
                tiles.append(t)
            return tiles
        wk_s = load_w(WkT, D, DG, "wk", nc.sync.dma_start)
        # x in 512-token pieces, token-major so K proj starts early
        x_s = []
        for dc in range(DC):
            xt = big.tile([P, T], bf16, tag=f"x{dc}")
            x_s.append(xt)
        for tcb in range(NQB):
            for dc in range(DC):
                nc.gpsimd.dma_start(
                    x_s[dc][:, tcb * BLK:(tcb + 1) * BLK],
                    xT[dc * P:(dc + 1) * P, tcb * BLK:(tcb + 1) * BLK])
        wv_s = load_w(WvT, D, DG, "wv", nc.scalar.dma_start)
        wq_s = load_w(WqT, D, DG, "wq", nc.scalar.dma_start)
        wo_s = load_w(WoT, DG, D, "wo", nc.sync.dma_start)

        # prime the pAB slots: triangle-cropped exp leaves the masked
        # columns unwritten, and 0 x (uninitialized inf/NaN) = NaN
        for _ in range(3):
            pz = ppool.tile([P, 2 * BLK], bf16, tag="pAB")
            nc.vector.memset(pz, 0.0)

        qfin = big.tile([P, OC, T], bf16, tag="qfin")
        kfin = big.tile([P, OC, T], bf16, tag="kfin")
        vaug = big.tile([P, NTT, 8, 66], bf16, tag="vaug")
        nc.vector.memset(vaug[:, :, :, 0:1], 1.0)
        nc.vector.memset(vaug[:, :, :, 65:66], 1.0)
        ctxn = big.tile([P, OC, T], bf16, tag="ctxn")

        def rope(fin, oc, t_lo, wid):
            # rotate fin[:, oc, t_lo:t_lo+wid] in place (one producer)
            sl = slice(t_lo, t_lo + wid)
            sw = rpool.tile([P, BLK], bf16, tag="sw")
            for (a, src) in ((0, 32), (32, 0), (64, 96), (96, 64)):
                nc.gpsimd.dma_start(sw[a:a + 32, :wid],
                                    fin[src:src + 32, oc, sl])
            t1 = rpool.tile([P, BLK], bf16, tag="t1")
            t2 = rpool.tile([P, BLK], bf16, tag="t2")
            nc.vector.tensor_mul(t1[:, :wid], fin[:, oc, sl], ck_s[:, sl])
            nc.vector.tensor_mul(t2[:, :wid], sw[:, :wid], sk_s[:, sl])
            nc.vector.tensor_add(fin[:, oc, sl], t1[:, :wid], t2[:, :wid])

        def proj_qk(fin, w_tiles, b_s, oc, tcb):
            # fin[:, oc, tcb*BLK:...] = (W^T x)[dims 128oc.., tokens]; rope
            ps = psmm.tile([P, BLK], f32, tag="mm")
            osl = slice(oc * P, (oc + 1) * P)
            tsl = slice(tcb * BLK, (tcb + 1) * BLK)
            for dc in range(DC):
                nc.tensor.matmul(ps, w_tiles[dc][:, osl], x_s[dc][:, tsl],
                                 start=(dc == 0),
                                 stop=(dc == DC - 1 and not use_bias))
            if use_bias:
                nc.tensor.matmul(ps, b_s[:, osl], ones512,
                                 start=False, stop=True)
            nc.vector.tensor_copy(fin[:, oc, tsl], ps)
            rope(fin, oc, tcb * BLK, BLK)

        # ---- K projection + RoPE (all tokens), V projection
        for tcb in range(NQB):
            for oc in range(OC):
                proj_qk(kfin, wk_s, bk_s if use_bias else None, oc, tcb)
        for tt in range(NTT):
            ps = psmm.tile([P, DG], f32, tag="mm")
            for dc in range(DC):
                nc.tensor.matmul(ps, x_s[dc][:, tt * P:(tt + 1) * P],
                                 wv_s[dc],
                                 start=(dc == 0),
                                 stop=(dc == DC - 1 and not use_bias))
            if use_bias:
                nc.tensor.matmul(ps, onesb, bv_s, start=False, stop=True)
            nc.vector.tensor_copy(vaug[:, tt, 0:8, 1:65], ps)

        # ---- main pipeline over query blocks
        for qb in range(NQB):
            for oc in range(OC):
                proj_qk(qfin, wq_s, bq_s if use_bias else None, oc, qb)
            qsl = slice(qb * BLK, (qb + 1) * BLK)
            J = 4 * qb + 4
            stg = npool.tile([8, BLK], f32, tag="stg")
            for oc in range(OC):
                opsA = psacc.tile([P, BLK], f32, tag="opsA")
                opsB = psacc.tile([P, BLK], f32, tag="opsB")
                for jt in range(J):
                    diag = jt >= 4 * qb
                    sAB = pssc.tile([P, 2 * BLK], f32, tag="sAB")
                    nc.tensor.matmul(
                        sAB[:, 0:BLK],
                        kfin[0:64, oc, jt * P:(jt + 1) * P],
                        qfin[0:64, oc, qsl],
                        start=True, stop=True, tile_position=(0, 0))
                    nc.tensor.matmul(
                        sAB[:, BLK:2 * BLK],
                        kfin[64:128, oc, jt * P:(jt + 1) * P],
                        qfin[64:128, oc, qsl],
                        start=True, stop=True, tile_position=(64, 0))
                    pAB = ppool.tile([P, 2 * BLK], bf16, tag="pAB")
                    if diag:
                        # triangle-crop: columns [0, 128*jl) of each half
                        # are fully masked — skip them in the exp
                        jl = jt - 4 * qb
                        if jl > 0:
                            co = jl * P
                            s_in = sAB[:, co:]
                            s_src = bass.AP(
                                tensor=s_in.tensor, offset=s_in.offset,
                                ap=[s_in.ap[0], [BLK, 2], [1, BLK - co]])
                            p_out = pAB[:, co:]
                            p_dst = bass.AP(
                                tensor=p_out.tensor, offset=p_out.offset,
                                ap=[p_out.ap[0], [BLK, 2], [1, BLK - co]])
                            nc.scalar.activation(p_dst, s_src, Exp,
                                                 scale=0.125)
                        else:
                            nc.scalar.activation(pAB, sAB, Exp, scale=0.125)
                        nc.vector.tensor_mul(pAB, pAB, mk_s[:, jl, :])
                    else:
                        nc.scalar.activation(pAB, sAB, Exp, scale=0.125)
                    nc.tensor.matmul(opsA[0:65, :],
                                     vaug[:, jt, 2 * oc, 1:66],
                                     pAB[:, 0:BLK],
                                     start=(jt == 0), stop=(jt == J - 1))
                    nc.tensor.matmul(opsB[0:65, :],
                                     vaug[:, jt, 2 * oc + 1, 1:66],
                                     pAB[:, BLK:2 * BLK],
                                     start=(jt == 0), stop=(jt == J - 1))
                # evacuate unnormalized ctx (releases opsA/B for next oc)
                # and stage the two denominator rows into stg rows 2oc,2oc+1
                nc.vector.tensor_copy(ctxn[0:64, oc, qsl], opsA[0:64, :])
                nc.vector.tensor_copy(ctxn[64:128, oc, qsl], opsB[0:64, :])
                for (hh, ops) in ((0, opsA), (1, opsB)):
                    dt = npool.tile([1, BLK], f32, tag="dtmp")
                    nc.vector.tensor_copy(dt, ops[64:65, :])
                    nc.sync.dma_start(stg[2 * oc + hh:2 * oc + hh + 1, :], dt)
            # batched reciprocal: rec = exp(-ln(den)), then per-oc broadcast
            lnd = npool.tile([8, BLK], f32, tag="lnd")
            nc.scalar.activation(lnd, stg, Ln)
            rec = npool.tile([8, BLK], bf16, tag="rec")
            nc.scalar.activation(rec, lnd, Exp, scale=-1.0)
            for oc in range(OC):
                bc = psmm.tile([P, BLK], f32, tag="mm")
                nc.tensor.matmul(bc, sel_s[:, oc, :], rec,
                                 start=True, stop=True)
                nc.vector.tensor_mul(ctxn[0:64, oc, qsl],
                                     ctxn[0:64, oc, qsl], bc[0:64, :])
                nc.vector.tensor_mul(ctxn[64:128, oc, qsl],
                                     ctxn[64:128, oc, qsl], bc[64:128, :])
            # output projection for this query block (partial: our heads)
            for tp in range(4):
                tsl = slice(qb * BLK + tp * P, qb * BLK + (tp + 1) * P)
                for half in range(2):
                    esl = slice(half * BLK, (half + 1) * BLK)
                    ps = psmm.tile([P, BLK], f32, tag="mm")
                    for oc in range(OC):
                        nc.tensor.matmul(ps, ctxn[:, oc, tsl],
                                         wo_s[oc][:, esl],
                                         start=(oc == 0),
                                         stop=(oc == OC - 1 and
                                               not use_bias))
                    if use_bias:
                        nc.tensor.matmul(ps, onesb, bo_s[:, esl],
                                         start=False, stop=True)
                    ot = outp.tile([P, BLK], f32, tag="ot")
                    nc.vector.tensor_copy(ot, ps)
                    nc.sync.dma_start(out_d[tsl, esl], ot)
    _legalize_waits(nc)
    return nc


# ------------------------------------------------------------------- entry

def kernel(x, Wq, bq, Wk, bk, Wv, bv, Wo, bo):
    x = np.asarray(x, np.float32)
    Wq, bq = np.asarray(Wq, np.float32), np.asarray(bq, np.float32)
    Wk, bk = np.asarray(Wk, np.float32), np.asarray(bk, np.float32)
    Wv, bv = np.asarray(Wv, np.float32), np.asarray(bv, np.float32)
    Wo, bo = np.asarray(Wo, np.float32), np.asarray(bo, np.float32)
    use_bias = bool(any(np.any(b) for b in (bq, bk, bv, bo)))
    in_maps = host_prep(x, Wq, bq, Wk, bk, Wv, bv, Wo, bo)
    if not use_bias:
        for m in in_maps:
            for k in ("bq", "bk", "bv", "bo"):
                m.pop(k)
    nc = build_nc(use_bias)
    res = run_bass_kernel_spmd(nc, in_maps, list(range(NCORES))).results
    return assemble(res)


# revision 21
# speedup vs baseline: 1.6281x; 1.2532x over previous
# Multi-head causal self-attention with RoPE on 8 NeuronCores (Trainium2).
#
# Sharding: TP-2 x DP-4, zero device communication. Core c handles batch
# b = c//2 and head group g = c%2 (heads 8g..8g+7) over ALL 2048 tokens.
# Each core computes a PARTIAL output projection (its 512 head-dims rows of
# Wo^T); the host sums the two partials per batch during unsharding.
# This removes the K/V-projection duplication of a pure-DP split and makes
# the causal tiling exact (no fully-masked score tiles).
#
# Pipeline (single phase, software-pipelined so Scalar exp overlaps PE):
#   K proj (+RoPE), V proj  ->  for qb in 0..3:
#     Q proj(qb) (+RoPE); for oc in 0..3: scores/exp/mask/attnV over the
#     causal j-tiles; per-(oc,qb) normalize via selector-matmul broadcast
#     of 1/den (no DRAM round trip); out-proj(qb) + output DMA.
#
# Layouts (on chip, bf16 compute / f32 accumulate):
#   q^T, k^T  [128 part = head-pair dims, tokens]    d-major for S^T matmuls
#   vaug      [128 part = tokens, 16 tt, 8 h, 66]    cols = [ones|V(64)|ones]
#             h-even uses cols 1:66 (ctx rows 0..63, den row 64);
#             h-odd  uses cols 0:65 (den row 63, ctx rows 64..127) so the
#             head-pair context lands lane-aligned in one 128-part tile.
#   S^T tiles [128 j-tokens, 1024] = h0|h1 halves    softmax along PARTITION
#             via matmul-with-ones; one exp instr covers both heads.
# RoPE uses an "evens-then-odds" permuted head layout (baked into Wq/Wk
# columns host-side) so the rotation partner is a fixed +-32 partition shift.

import sys

import numpy as np
import ml_dtypes

for _p in ("/opt/trn_rl_repo",):
    try:
        import concourse.bass  # noqa: F401
        break
    except ImportError:
        sys.path.insert(0, _p)

import concourse.bass as bass
import concourse.tile as tile
from concourse import mybir
from concourse.bass_utils import run_bass_kernel_spmd

B, T, D, H, DH = 4, 2048, 1024, 16, 64
THETA = 10000.0
NCORES = 8
P = 128
DG = 512   # head dims per core (8 heads)
OC = 4     # 128-wide head-pair chunks per core
DC = 8     # 128-wide input-dim chunks
BLK = 512  # query block width
NQB = 4    # query blocks
NTT = 16   # 128-token tiles

f32 = mybir.dt.float32
bf16 = mybir.dt.bfloat16
BF = ml_dtypes.bfloat16


# ---------------------------------------------------------------- host prep

def _perm():
    """Column permutation: within each head's 64 dims, evens then odds."""
    p = np.empty(D, np.int64)
    for h in range(H):
        for m in range(32):
            p[h * 64 + m] = h * 64 + 2 * m
            p[h * 64 + 32 + m] = h * 64 + 2 * m + 1
    return p


def _rope_tables():
    """cos/sin tables [128, T] for the permuted (evens-first) layout."""
    inv = THETA ** (-(np.arange(0, DH, 2, dtype=np.float64) / DH))  # [32]
    m = np.arange(P) % 64
    fi = m % 32
    pos = np.arange(T)
    ang = pos[None, :].astype(np.float64) * inv[fi][:, None]  # [128, T]
    cos = np.cos(ang)
    sin = np.sin(ang) * np.where(m < 64 // 2, -1.0, 1.0)[:, None]
    return cos.astype(np.float32), sin.astype(np.float32)


def _masks():
    """Staircase masks [4, 128, 1024] (identical h0|h1 halves).

    For the diagonal j-tile at kv offset 128*jt within a 512-query block:
    m[jt, r, c] = 1 iff 128*jt + r <= (c % 512).
    """
    mk = np.zeros((4, P, 2 * BLK), np.float32)
    r = np.arange(P)[:, None]
    cl = (np.arange(2 * BLK) % BLK)[None, :]
    for jt in range(4):
        mk[jt] = (128 * jt + r <= cl).astype(np.float32)
    return mk


def host_prep(x, Wq, bq, Wk, bk, Wv, bv, Wo, bo):
    """Build the 8 per-core input dicts (numpy, bf16)."""
    perm = _perm()
    WqTp = np.ascontiguousarray(Wq.T[:, perm]).astype(BF)
    WkTp = np.ascontiguousarray(Wk.T[:, perm]).astype(BF)
    WvT = np.ascontiguousarray(Wv.T).astype(BF)
    WoT = np.ascontiguousarray(Wo.T).astype(BF)
    bqp = bq[perm].astype(np.float32)
    bkp = bk[perm].astype(np.float32)
    ck, sk = _rope_tables()
    mk = _masks().reshape(4 * P, 2 * BLK)
    # sel[r, oc, c]: broadcast-selector for the per-(oc) 1/den matmul:
    # row 2oc -> cols 0:64 (head h0), row 2oc+1 -> cols 64:128 (head h1)
    sel = np.zeros((8, OC, P), np.float32)
    for oc in range(OC):
        sel[2 * oc, oc, 0:64] = 1.0
        sel[2 * oc + 1, oc, 64:128] = 1.0
    sel = sel.reshape(8, OC * P)
    in_maps = []
    for c in range(NCORES):
        b, g = c // 2, c % 2
        gs = slice(DG * g, DG * (g + 1))
        in_maps.append({
            "xT": np.ascontiguousarray(x[b].T).astype(BF),
            "WqT": WqTp[:, gs], "WkT": WkTp[:, gs],
            "WvT": np.ascontiguousarray(WvT[:, gs]),
            "WoT": np.ascontiguousarray(WoT[gs, :]),
            "bq": bqp[gs].reshape(1, DG).astype(BF),
            "bk": bkp[gs].reshape(1, DG).astype(BF),
            "bv": bv[gs].reshape(1, DG).astype(BF),
            # host sums the two partials, so each adds half of bo
            "bo": (0.5 * bo).reshape(1, D).astype(BF),
            "ck": ck.astype(BF), "sk": sk.astype(BF),
            "mk": mk.astype(BF), "sel": sel.astype(BF),
        })
    return in_maps


def assemble(results):
    y = np.empty((B, T, D), np.float32)
    for b in range(B):
        y[b] = results[2 * b]["out"] + results[2 * b + 1]["out"]
    return y


# ------------------------------------------------------------- device build

def _legalize_waits(nc, max_waits=1):
    """Limit every instruction to one sync-wait command.

    Walrus's per-instruction structs encode a single sync wait; Tile can
    emit more. For any instruction with k > 1 waits, insert k-1 nops on
    the same engine immediately before it, each carrying one wait —
    position-preserving, so semantics are unchanged.
    """
    eng_obj = {
        mybir.EngineType.PE: nc.tensor,
        mybir.EngineType.Activation: nc.scalar,
        mybir.EngineType.DVE: nc.vector,
        mybir.EngineType.Pool: nc.gpsimd,
        mybir.EngineType.SP: nc.sync,
    }
    fn = nc.m.functions[0]
    for blk in fn.blocks:
        insts = list(blk.instructions)
        new = []
        for inst in insts:
            si = inst.sync_info
            nw = len(si.on_wait) if si is not None else 0
            if nw > max_waits:
                for w in si.on_wait[: nw - max_waits]:
                    eng_obj[inst.engine].nop()
                    nop = fn.blocks[-1].instructions[-1]
                    fn.blocks[-1].instructions = \
                        fn.blocks[-1].instructions[:-1]
                    nop.sync_info = mybir.SyncInfo(on_wait=[w], on_update=[])
                    new.append(nop)
                inst.sync_info = mybir.SyncInfo(
                    on_wait=list(si.on_wait[nw - max_waits:]),
                    on_update=list(si.on_update))
            new.append(inst)
        blk.instructions = new


def build_nc(use_bias):
    from contextlib import ExitStack

    nc = bass.Bass("TRN2", target_bir_lowering=False, debug=False,
                   num_devices=NCORES)
    Exp = mybir.ActivationFunctionType.Exp
    Ln = mybir.ActivationFunctionType.Ln

    xT = nc.dram_tensor("xT", [D, T], bf16, kind="ExternalInput").ap()
    WqT = nc.dram_tensor("WqT", [D, DG], bf16, kind="ExternalInput").ap()
    WkT = nc.dram_tensor("WkT", [D, DG], bf16, kind="ExternalInput").ap()
    WvT = nc.dram_tensor("WvT", [D, DG], bf16, kind="ExternalInput").ap()
    WoT = nc.dram_tensor("WoT", [DG, D], bf16, kind="ExternalInput").ap()
    if use_bias:
        bq_d = nc.dram_tensor("bq", [1, DG], bf16, kind="ExternalInput").ap()
        bk_d = nc.dram_tensor("bk", [1, DG], bf16, kind="ExternalInput").ap()
        bv_d = nc.dram_tensor("bv", [1, DG], bf16, kind="ExternalInput").ap()
        bo_d = nc.dram_tensor("bo", [1, D], bf16, kind="ExternalInput").ap()
    ck_d = nc.dram_tensor("ck", [P, T], bf16, kind="ExternalInput").ap()
    sk_d = nc.dram_tensor("sk", [P, T], bf16, kind="ExternalInput").ap()
    mk_d = nc.dram_tensor("mk", [4 * P, 2 * BLK], bf16,
                          kind="ExternalInput").ap()
    sel_d = nc.dram_tensor("sel", [8, OC * P], bf16,
                           kind="ExternalInput").ap()
    out_d = nc.dram_tensor("out", [T, D], f32, kind="ExternalOutput").ap()

    with tile.TileContext(nc) as tc, ExitStack() as ctx:
        big = ctx.enter_context(tc.tile_pool(name="big", bufs=1))
        const = ctx.enter_context(tc.tile_pool(name="const", bufs=1))
        rpool = ctx.enter_context(tc.tile_pool(name="rp", bufs=2))
        ppool = ctx.enter_context(tc.tile_pool(name="pp", bufs=3))
        npool = ctx.enter_context(tc.tile_pool(name="np", bufs=2))
        outp = ctx.enter_context(tc.tile_pool(name="outp", bufs=3))
        pssc = ctx.enter_context(
            tc.tile_pool(name="pssc", bufs=2, space="PSUM"))
        psacc = ctx.enter_context(
            tc.tile_pool(name="psacc", bufs=1, space="PSUM"))
        psmm = ctx.enter_context(
            tc.tile_pool(name="psmm", bufs=2, space="PSUM"))

        # ---- constants (small first, scalar queue is idle early)
        sel_s = const.tile([8, OC, P], bf16, tag="sel")
        nc.scalar.dma_start(sel_s, sel_d.rearrange("r (oc p) -> r oc p", p=P))
        mk_s = const.tile([P, 4, 2 * BLK], bf16, tag="mk")
        nc.scalar.dma_start(
            mk_s, mk_d.rearrange("(jt p) i -> p jt i", p=P))
        ck_s = const.tile([P, T], bf16, tag="ck")
        nc.scalar.dma_start(ck_s, ck_d)
        sk_s = const.tile([P, T], bf16, tag="sk")
        nc.scalar.dma_start(sk_s, sk_d)
        if use_bias:
            bq_s = const.tile([1, DG], bf16, tag="bq")
            nc.scalar.dma_start(bq_s, bq_d)
            bk_s = const.tile([1, DG], bf16, tag="bk")
            nc.scalar.dma_start(bk_s, bk_d)
            bv_s = const.tile([1, DG], bf16, tag="bv")
            nc.scalar.dma_start(bv_s, bv_d)
            bo_s = const.tile([1, D], bf16, tag="bo")
            nc.scalar.dma_start(bo_s, bo_d)
            ones512 = const.tile([1, BLK], bf16, tag="ones512")
            nc.vector.memset(ones512, 1.0)
            onesb = const.tile([1, P], bf16, tag="onesb")
            nc.vector.memset(onesb, 1.0)

        # ---- weights: wk first (K proj runs first), then others
        def load_w(src, n_in, n_col, tagp, q):
            tiles = []
            for dc in range(n_in // P):
                t = big.tile([P, n_col], bf16, tag=f"{tagp}{dc}")
                q(t, src[dc * P:(dc + 1) * P, :])
                tiles.append(t)
            return tiles
        wk_s = load_w(WkT, D, DG, "wk", nc.sync.dma_start)
        # x in 512-token pieces, token-major so K proj starts early;
        # alternate queues so the ~600ns trigger cost doesn't serialize
        x_s = []
        for dc in range(DC):
            xt = big.tile([P, T], bf16, tag=f"x{dc}")
            x_s.append(xt)
        for tcb in range(NQB):
            for dc in range(DC):
                q = nc.gpsimd.dma_start if dc % 2 else nc.sync.dma_start
                q(x_s[dc][:, tcb * BLK:(tcb + 1) * BLK],
                  xT[dc * P:(dc + 1) * P, tcb * BLK:(tcb + 1) * BLK])
        wv_s = load_w(WvT, D, DG, "wv", nc.scalar.dma_start)
        wq_s = load_w(WqT, D, DG, "wq", nc.scalar.dma_start)
        wo_s = load_w(WoT, DG, D, "wo", nc.sync.dma_start)

        # prime the pAB slots: triangle-cropped exp leaves the masked
        # columns unwritten, and 0 x (uninitialized inf/NaN) = NaN
        for _ in range(3):
            pz = ppool.tile([P, 2 * BLK], bf16, tag="pAB")
            nc.vector.memset(pz, 0.0)

        qfin = big.tile([P, OC, T], bf16, tag="qfin")
        kfin = big.tile([P, OC, T], bf16, tag="kfin")
        vaug = big.tile([P, NTT, 8, 66], bf16, tag="vaug")
        nc.vector.memset(vaug[:, :, :, 0:1], 1.0)
        nc.vector.memset(vaug[:, :, :, 65:66], 1.0)
        ctxn = big.tile([P, OC, T], bf16, tag="ctxn")

        def rope(fin, oc, t_lo, wid):
            # rotate fin[:, oc, t_lo:t_lo+wid] in place (one producer)
            sl = slice(t_lo, t_lo + wid)
            sw = rpool.tile([P, T], bf16, tag="sw")
            for (a, src) in ((0, 32), (32, 0), (64, 96), (96, 64)):
                nc.gpsimd.dma_start(sw[a:a + 32, :wid],
                                    fin[src:src + 32, oc, sl])
            t1 = rpool.tile([P, T], bf16, tag="t1")
            t2 = rpool.tile([P, T], bf16, tag="t2")
            nc.vector.tensor_mul(t1[:, :wid], fin[:, oc, sl], ck_s[:, sl])
            nc.vector.tensor_mul(t2[:, :wid], sw[:, :wid], sk_s[:, sl])
            nc.vector.tensor_add(fin[:, oc, sl], t1[:, :wid], t2[:, :wid])

        def proj_qk(fin, w_tiles, b_s, oc, tcb, rope_now=True):
            # fin[:, oc, tcb*BLK:...] = (W^T x)[dims 128oc.., tokens]
            ps = psmm.tile([P, BLK], f32, tag="mm")
            osl = slice(oc * P, (oc + 1) * P)
            tsl = slice(tcb * BLK, (tcb + 1) * BLK)
            for dc in range(DC):
                nc.tensor.matmul(ps, w_tiles[dc][:, osl], x_s[dc][:, tsl],
                                 start=(dc == 0),
                                 stop=(dc == DC - 1 and not use_bias))
            if use_bias:
                nc.tensor.matmul(ps, b_s[:, osl], ones512,
                                 start=False, stop=True)
            nc.vector.tensor_copy(fin[:, oc, tsl], ps)
            if rope_now:
                rope(fin, oc, tcb * BLK, BLK)

        # ---- K projection (all tokens) + batched RoPE, V projection
        for tcb in range(NQB):
            for oc in range(OC):
                proj_qk(kfin, wk_s, bk_s if use_bias else None, oc, tcb,
                        rope_now=False)
        for oc in range(OC):
            rope(kfin, oc, 0, T)
        for tt in range(NTT):
            ps = psmm.tile([P, DG], f32, tag="mm")
            for dc in range(DC):
                nc.tensor.matmul(ps, x_s[dc][:, tt * P:(tt + 1) * P],
                                 wv_s[dc],
                                 start=(dc == 0),
                                 stop=(dc == DC - 1 and not use_bias))
            if use_bias:
                nc.tensor.matmul(ps, onesb, bv_s, start=False, stop=True)
            nc.vector.tensor_copy(vaug[:, tt, 0:8, 1:65], ps)

        def strip(ap2d, co):
            # cols [co:co+128] and [BLK+co:BLK+co+128] of a [128, 2*BLK] AP
            s = ap2d[:, co:]
            return bass.AP(tensor=s.tensor, offset=s.offset,
                           ap=[s.ap[0], [BLK, 2], [1, P]])

        # ---- main pipeline over query blocks
        for oc in range(OC):
            proj_qk(qfin, wq_s, bq_s if use_bias else None, oc, 0)
        for qb in range(NQB):
            qsl = slice(qb * BLK, (qb + 1) * BLK)
            J = 4 * qb + 4
            stg = npool.tile([8, BLK], f32, tag="stg")
            for oc in range(OC):
                opsA = psacc.tile([P, BLK], f32, tag="opsA")
                opsB = psacc.tile([P, BLK], f32, tag="opsB")
                for jt in range(J):
                    # diagonal tiles: queries [0, co) of this block can't
                    # see kv tile jt — shrink every op to cols [co, BLK)
                    jl = jt - 4 * qb
                    co = jl * P if jl > 0 else 0
                    qco = slice(qb * BLK + co, (qb + 1) * BLK)
                    sAB = pssc.tile([P, 2 * BLK], f32, tag="sAB")
                    nc.tensor.matmul(
                        sAB[:, co:BLK],
                        kfin[0:64, oc, jt * P:(jt + 1) * P],
                        qfin[0:64, oc, qco],
                        start=True, stop=True, tile_position=(0, 0))
                    nc.tensor.matmul(
                        sAB[:, BLK + co:2 * BLK],
                        kfin[64:128, oc, jt * P:(jt + 1) * P],
                        qfin[64:128, oc, qco],
                        start=True, stop=True, tile_position=(64, 0))
                    pAB = ppool.tile([P, 2 * BLK], bf16, tag="pAB")
                    if co > 0:
                        s_in = sAB[:, co:]
                        s_src = bass.AP(
                            tensor=s_in.tensor, offset=s_in.offset,
                            ap=[s_in.ap[0], [BLK, 2], [1, BLK - co]])
                        p_out = pAB[:, co:]
                        p_dst = bass.AP(
                            tensor=p_out.tensor, offset=p_out.offset,
                            ap=[p_out.ap[0], [BLK, 2], [1, BLK - co]])
                        nc.scalar.activation(p_dst, s_src, Exp, scale=0.125)
                    else:
                        nc.scalar.activation(pAB, sAB, Exp, scale=0.125)
                    if jl >= 0:
                        # staircase mask only on the 128-wide diagonal strip
                        nc.vector.tensor_mul(strip(pAB, co), strip(pAB, co),
                                             strip(mk_s[:, jl, :], co))
                    nc.tensor.matmul(opsA[0:65, co:BLK],
                                     vaug[:, jt, 2 * oc, 1:66],
                                     pAB[:, co:BLK],
                                     start=(jt == 0), stop=(jt == J - 1))
                    nc.tensor.matmul(opsB[0:65, co:BLK],
                                     vaug[:, jt, 2 * oc + 1, 1:66],
                                     pAB[:, BLK + co:2 * BLK],
                                     start=(jt == 0), stop=(jt == J - 1))
                # evacuate unnormalized ctx (releases opsA/B for next oc)
                # and stage the two denominator rows into stg rows 2oc,2oc+1
                nc.vector.tensor_copy(ctxn[0:64, oc, qsl], opsA[0:64, :])
                nc.vector.tensor_copy(ctxn[64:128, oc, qsl], opsB[0:64, :])
                for (hh, ops) in ((0, opsA), (1, opsB)):
                    dt = npool.tile([1, BLK], f32, tag="dtmp")
                    nc.vector.tensor_copy(dt, ops[64:65, :])
                    nc.sync.dma_start(stg[2 * oc + hh:2 * oc + hh + 1, :], dt)
            # Q projection for the next block — keeps PE busy while the
            # normalize chain (stage DMA -> ln -> exp -> bcast) drains
            if qb < NQB - 1:
                for oc in range(OC):
                    proj_qk(qfin, wq_s, bq_s if use_bias else None,
                            oc, qb + 1)
            # batched reciprocal: rec = exp(-ln(den)), then per-oc broadcast
            lnd = npool.tile([8, BLK], f32, tag="lnd")
            nc.scalar.activation(lnd, stg, Ln)
            rec = npool.tile([8, BLK], bf16, tag="rec")
            nc.scalar.activation(rec, lnd, Exp, scale=-1.0)
            for oc in range(OC):
                bc = psmm.tile([P, BLK], f32, tag="mm")
                nc.tensor.matmul(bc, sel_s[:, oc, :], rec,
                                 start=True, stop=True)
                nc.vector.tensor_mul(ctxn[0:64, oc, qsl],
                                     ctxn[0:64, oc, qsl], bc[0:64, :])
                nc.vector.tensor_mul(ctxn[64:128, oc, qsl],
                                     ctxn[64:128, oc, qsl], bc[64:128, :])
            # output projection for this query block (partial: our heads)
            for tp in range(4):
                tsl = slice(qb * BLK + tp * P, qb * BLK + (tp + 1) * P)
                for half in range(2):
                    esl = slice(half * BLK, (half + 1) * BLK)
                    ps = psmm.tile([P, BLK], f32, tag="mm")
                    for oc in range(OC):
                        nc.tensor.matmul(ps, ctxn[:, oc, tsl],
                                         wo_s[oc][:, esl],
                                         start=(oc == 0),
                                         stop=(oc == OC - 1 and
                                               not use_bias))
                    if use_bias:
                        nc.tensor.matmul(ps, onesb, bo_s[:, esl],
                                         start=False, stop=True)
                    ot = outp.tile([P, BLK], f32, tag="ot")
                    nc.vector.tensor_copy(ot, ps)
                    nc.sync.dma_start(out_d[tsl, esl], ot)
    _legalize_waits(nc)
    return nc


# ------------------------------------------------------------------- entry

def kernel(x, Wq, bq, Wk, bk, Wv, bv, Wo, bo):
    x = np.asarray(x, np.float32)
    Wq, bq = np.asarray(Wq, np.float32), np.asarray(bq, np.float32)
    Wk, bk = np.asarray(Wk, np.float32), np.asarray(bk, np.float32)
    Wv, bv = np.asarray(Wv, np.float32), np.asarray(bv, np.float32)
    Wo, bo = np.asarray(Wo, np.float32), np.asarray(bo, np.float32)
    use_bias = bool(any(np.any(b) for b in (bq, bk, bv, bo)))
    in_maps = host_prep(x, Wq, bq, Wk, bk, Wv, bv, Wo, bo)
    if not use_bias:
        for m in in_maps:
            for k in ("bq", "bk", "bv", "bo"):
                m.pop(k)
    nc = build_nc(use_bias)
    res = run_bass_kernel_spmd(nc, in_maps, list(range(NCORES))).results
    return assemble(res)
